# revision 8
# baseline (speedup 1.0000x reference)
import numpy as np

import concourse.bacc as bacc
import concourse.bass as bass
import concourse.mybir as mybir
import concourse.tile as tile
from concourse.bass_utils import run_bass_kernel_spmd

CLAMP_LO, CLAMP_HI = -123.68, 151.061
C, TEX = 16, 1024
B, H, W = 4, 768, 768
N_CORES = 8
ROWS = H // N_CORES          # 96 rows per core
PIX = ROWS * W               # 73728 pixels per plane per core
P = 128                      # SBUF partitions
S = PIX // P                 # 576 pixels per partition
NCHUNK = 8
K = S // NCHUNK              # 72 indices per partition per gather
HALF = S // 2                # output tile split point (288 pixels)
F32 = mybir.dt.float32
I32 = mybir.dt.int32

_prog = None
last_results = None


def _build_program(rep=1):
    nc = bacc.Bacc()
    x_in = nc.declare_dram_parameter("x", [B, PIX, 2], F32, isOutput=False)
    tb_in = nc.declare_dram_parameter("table", [TEX * TEX, C * 4], F32, isOutput=False)
    out_ext = nc.declare_dram_parameter("out", [B, C, PIX], F32, isOutput=True)

    add = mybir.AluOpType.add
    mult = mybir.AluOpType.mult
    sub = mybir.AluOpType.subtract
    amin = mybir.AluOpType.min
    amax = mybir.AluOpType.max
    is_gt = mybir.AluOpType.is_gt
    X_AX = mybir.AxisListType.X

    with tile.TileContext(nc) as tc:
        with tc.tile_pool(name="main", bufs=1) as pool:
          for _r in range(rep):
            idx_tiles = []
            w_tiles = []
            for b in range(B):
                X = pool.tile([P, 2 * S], F32, name=f"X{b}", tag="X", bufs=2)
                nc.sync.dma_start(
                    out=X[:], in_=x_in[b].rearrange("(p s) t -> p (s t)", p=P)
                )
                Xv = X[:].rearrange("p (s t) -> p s t", t=2)

                Wt = pool.tile([P, 4 * S], F32, name=f"Wt{b}", tag=f"Wt{b}", bufs=1)
                IDX = pool.tile([P, S], I32, name=f"IDX{b}", tag=f"IDX{b}", bufs=1)
                Wv = Wt[:].rearrange("p (s t) -> p s t", t=4)

                cpt = {}
                for ax in (0, 1):  # 0: x-coord, 1: y-coord
                    IC = pool.tile([P, S], F32, name=f"IC{b}{ax}", tag=f"IC{ax}", bufs=1)
                    XI = pool.tile([P, S], I32, name=f"XI{b}{ax}", tag=f"XI{ax}", bufs=1)
                    X0 = pool.tile([P, S], F32, name=f"X0{b}{ax}", tag=f"X0{ax}", bufs=1)
                    CR = pool.tile([P, S], F32, name=f"CR{b}{ax}", tag=f"CR{ax}", bufs=1)
                    WX = pool.tile([P, S], F32, name=f"WX{b}{ax}", tag=f"WX{ax}", bufs=1)
                    UX = pool.tile([P, S], F32, name=f"UX{b}{ax}", tag=f"UX{ax}", bufs=1)
                    # ic = clamp((x + 1) * 511.5, 0, 1023); bit-identical to
                    # ((x+1)*0.5)*1023 since *0.5 is exact
                    nc.vector.tensor_scalar(
                        out=IC[:], in0=Xv[:, :, ax],
                        scalar1=1.0, scalar2=511.5, op0=add, op1=mult,
                    )
                    nc.vector.tensor_scalar(
                        out=IC[:], in0=IC[:],
                        scalar1=0.0, scalar2=1023.0, op0=amax, op1=amin,
                    )
                    # floor(ic), robust to either trunc or round-nearest f32->i32
                    nc.vector.tensor_copy(out=XI[:], in_=IC[:])
                    nc.vector.tensor_copy(out=X0[:], in_=XI[:])
                    nc.vector.tensor_tensor(out=CR[:], in0=X0[:], in1=IC[:], op=is_gt)
                    nc.vector.tensor_tensor(out=X0[:], in0=X0[:], in1=CR[:], op=sub)
                    nc.vector.tensor_tensor(out=WX[:], in0=IC[:], in1=X0[:], op=sub)
                    # ux = 1 - wx
                    nc.vector.tensor_scalar(
                        out=UX[:], in0=WX[:],
                        scalar1=-1.0, scalar2=1.0, op0=mult, op1=add,
                    )
                    cpt[ax] = (X0, WX, UX)

                X0, WXt, UXt = cpt[0]
                Y0, WYt, UYt = cpt[1]
                # per-pixel interleaved weights [s, 4] = w00, w01, w10, w11
                nc.vector.tensor_tensor(out=Wv[:, :, 0], in0=UXt[:], in1=UYt[:], op=mult)
                nc.vector.tensor_tensor(out=Wv[:, :, 1], in0=WXt[:], in1=UYt[:], op=mult)
                nc.vector.tensor_tensor(out=Wv[:, :, 2], in0=UXt[:], in1=WYt[:], op=mult)
                nc.vector.tensor_tensor(out=Wv[:, :, 3], in0=WXt[:], in1=WYt[:], op=mult)
                # idx = y0 * 1024 + x0 (integral, < 2^24: exact in f32)
                IDXF = pool.tile([P, S], F32, name=f"IDXF{b}", tag="IDXF", bufs=1)
                nc.vector.scalar_tensor_tensor(
                    out=IDXF[:], in0=Y0[:], scalar=1024.0, in1=X0[:], op0=mult, op1=add
                )
                nc.vector.tensor_copy(out=IDX[:], in_=IDXF[:])
                idx_tiles.append(IDX)
                w_tiles.append(Wv)

            # gather + weighted-sum phase: 32 back-to-back gathers
            for b in range(B):
                IDX = idx_tiles[b]
                Wv = w_tiles[b]
                for h in range(2):
                    OT = pool.tile([P, C * HALF], F32, name=f"OT{b}{h}", tag="OT", bufs=2)
                    OTv = OT[:].rearrange("p (c s) -> p s c", c=C)
                    for jj in range(NCHUNK // 2):
                        j = h * (NCHUNK // 2) + jj
                        G = pool.tile([P, K * C * 4], F32, name=f"G{b}{j}", tag="G", bufs=2)
                        for j2 in range(K):
                            col = j * K + j2
                            nc.gpsimd.indirect_dma_start(
                                out=G[:, j2 * C * 4:(j2 + 1) * C * 4],
                                out_offset=None,
                                in_=tb_in[:],
                                in_offset=bass.IndirectOffsetOnAxis(
                                    ap=IDX[:, col:col + 1], axis=0
                                ),
                            )
                        Gv = G[:].rearrange("p (i c t) -> p i c t", c=C, t=4)
                        Wc = Wv[:, j * K:(j + 1) * K, None, :].to_broadcast([P, K, C, 4])
                        nc.vector.tensor_tensor(out=Gv, in0=Gv, in1=Wc, op=mult)
                        nc.vector.tensor_reduce(
                            out=OTv[:, jj * K:(jj + 1) * K, :], in_=Gv, axis=X_AX, op=add
                        )
                    # out[b, c, p*576 + h*288 + s] = OT[p, c*288 + s]
                    dst = out_ext[b].rearrange("c (p s) -> p c s", p=P)
                    nc.sync.dma_start(
                        out=dst[:, :, h * HALF:(h + 1) * HALF],
                        in_=OT[:].rearrange("p (c s) -> p c s", c=C),
                    )
    nc.finalize()
    return nc


def _get_program():
    global _prog
    if _prog is None:
        _prog = _build_program()
    return _prog


def _build_table(data: np.ndarray) -> np.ndarray:
    tex = np.clip(data, CLAMP_LO, CLAMP_HI)  # [C, T, T]
    xp1 = np.concatenate([tex[:, :, 1:], tex[:, :, -1:]], axis=2)
    yp1 = np.concatenate([tex[:, 1:, :], tex[:, -1:, :]], axis=1)
    xyp1 = np.concatenate([xp1[:, 1:, :], xp1[:, -1:, :]], axis=1)
    # table[y*T+x, c*4+t]; t = (y0,x0),(y0,x1),(y1,x0),(y1,x1), border-clamped
    table = np.stack([tex, xp1, yp1, xyp1], axis=-1)  # [C, T, T, 4]
    table = np.ascontiguousarray(table.transpose(1, 2, 0, 3)).reshape(TEX * TEX, C * 4)
    return table


def kernel(x: np.ndarray, data: np.ndarray) -> np.ndarray:
    x = np.ascontiguousarray(x, dtype=np.float32)
    data = np.ascontiguousarray(data, dtype=np.float32)
    table = _build_table(data)

    core_ids = list(range(N_CORES))
    in_maps = []
    for i in core_ids:
        xs = np.ascontiguousarray(x[:, i * ROWS:(i + 1) * ROWS]).reshape(B, PIX, 2)
        in_maps.append({"x": xs, "table": table})

    nc = _get_program()
    res = run_bass_kernel_spmd(nc, in_maps, core_ids)
    global last_results
    last_results = res

    out = np.empty((B, C, H, W), dtype=np.float32)
    for i in core_ids:
        out[:, :, i * ROWS:(i + 1) * ROWS, :] = (
            res.results[i]["out"].reshape(B, C, ROWS, W)
        )
    return out
